# revision 5
# baseline (speedup 1.0000x reference)
"""Trainium2 Bass kernel for nn_Block_627065225827 (dense_transformer).

Self-contained: hardcodes shapes B=32, T=4096, C=256, H=8 and the
data-parallel-over-batch sharding (4 batch rows per core, 8 cores).

Math (see reference):
    h   = LN1(x) * g1 + b1ln
    id  = h @ w_id.T ;  inf = h @ w_inf.T            (per-head view [H, hs])
    inf = inf / (1+K);  shifted[t] = inf[t - s_h]    (zero for t < s_h)
    sa  = (K/(1+K) * id + shifted) @ w_proj.T + b_proj
    x1  = x + sa
    ff  = relu(LN2(x1)*g2+b2ln @ w1.T + b1) @ w2.T + b2
    out = x1 + ff

Host-side algebraic folding (exact):
    sa[t] = w_a @ xhat[t] + sum_s W_s @ xhat[t-s] + const_rows
      w_a = w_proj @ (diag(a_row) @ (w_id * g1))           a_h = K/(1+K)
      W_s = w_proj[:, cols_s] @ ((w_inf * g1) * binv)[cols_s, :]
    so the per-head temporal shift becomes a free-dim offset into the
    transposed activation buffer hB (channels on partitions, tokens on
    free dim), with 4 zero pad columns implementing the t<s mask.
"""

import os
from contextlib import ExitStack

import numpy as np
import ml_dtypes

B, T, C, H = 32, 4096, 256, 8
HS = C // H
NCORES = 8
BPC = B // NCORES  # batch rows per core
SHIFTS = [1, 2, 3, 4, 1, 2, 3, 4]
EPS = 1e-5
PAD = 4  # zero columns at the head of hB for the shift mask
WIN = 512  # tokens per window
SUB = 128  # tokens per subtile (partition dim)

_f64 = np.float64
_bf16 = ml_dtypes.bfloat16
_fp8 = ml_dtypes.float8_e4m3
UP_W_SCALE = 16.0


def _prep(inputs):
    """Fold LN gains/biases + per-head scalars into the weights (host, numpy)."""
    g = {k: np.asarray(v, dtype=_f64) for k, v in inputs.items() if k != "x"}
    K = np.exp(g["khead"])  # [H]
    a_row = np.repeat(K / (1.0 + K), HS)  # [C] per id-output channel
    b_row = np.repeat(1.0 / (1.0 + K), HS)  # [C] per inf-output channel

    w_id_g = g["w_id"] * g["ln1_g"][None, :]
    w_inf_g = g["w_inf"] * g["ln1_g"][None, :]
    w_id_s = w_id_g * a_row[:, None]
    w_inf_s = w_inf_g * b_row[:, None]

    w_a = g["w_proj"] @ w_id_s  # [C, C]
    wsT = np.zeros((4, C, C), _f64)
    c_s = np.zeros((4, C), _f64)
    cid = w_id_g @ g["ln1_b"]
    cinf = w_inf_g @ g["ln1_b"]
    for s in range(1, 5):
        cols = np.concatenate(
            [np.arange(h * HS, (h + 1) * HS) for h in range(H) if SHIFTS[h] == s]
        )
        wsT[s - 1] = (g["w_proj"][:, cols] @ w_inf_s[cols, :]).T
        c_s[s - 1] = g["w_proj"][:, cols] @ (b_row * cinf)[cols]
    c_a = g["w_proj"] @ (a_row * cid) + g["b_proj"]  # [C]

    w1_g = g["w1"] * g["ln2_g"][None, :]
    b1_eff = g["w1"] @ g["ln2_b"] + g["b1"]  # [4C]
    w2T = g["w2"].T  # [4C, C]
    b2_eff = g["b2"]  # [C]

    # fp8 up-projection: scale weights x16 (keeps N(0,.02) entries out of the
    # subnormal range), compensated exactly by relu's input scale (1/16).
    w1dr = (w1_g.T * UP_W_SCALE).reshape(2, 128, 4 * C).transpose(1, 0, 2)
    out = {
        "w_aT": np.ascontiguousarray(w_a.T.reshape(2, 128, C)).astype(_bf16),
        "wsT": np.ascontiguousarray(wsT.reshape(4, 2, 128, C)).astype(_bf16),
        "w1dr": np.ascontiguousarray(w1dr).astype(_fp8),
        "w2T": np.ascontiguousarray(w2T.reshape(8, 128, C)).astype(_bf16),
        "b1c": np.ascontiguousarray(b1_eff.reshape(8, 128).T).astype(np.float32),
    }
    crows_attn = np.stack([c_a, c_s[0], c_s[1], c_s[2], c_s[3]])  # [5, C]
    use_crows = bool(np.abs(crows_attn).max() > 0)
    use_b2row = bool(np.abs(b2_eff).max() > 0)
    if use_crows or use_b2row:
        out["crows"] = np.concatenate([crows_attn, b2_eff[None, :]]).reshape(1, 6 * C).astype(_bf16)
    return out, use_crows, use_b2row


def _build(n_rows=BPC, t_len=T, use_crows=False, use_b2row=False):
    """Build + compile the per-core Bass program. Returns the finalized nc."""
    import concourse.bacc as bacc
    import concourse.bass as bass
    import concourse.mybir as mybir
    import concourse.tile as tile
    from concourse.masks import make_identity

    dt = mybir.dt
    AF = mybir.ActivationFunctionType
    OP = mybir.AluOpType

    nwin = t_len // WIN
    nc = bacc.Bacc("TRN2", target_bir_lowering=False, debug=False, num_devices=NCORES)

    x_d = nc.declare_dram_parameter("x", [n_rows, t_len, C], dt.float32, isOutput=False)
    waT_d = nc.declare_dram_parameter("w_aT", [2, 128, C], dt.bfloat16, isOutput=False)
    wsT_d = nc.declare_dram_parameter("wsT", [4, 2, 128, C], dt.bfloat16, isOutput=False)
    w1dr_d = nc.declare_dram_parameter("w1dr", [128, 2, 4 * C], dt.float8e4, isOutput=False)
    w2T_d = nc.declare_dram_parameter("w2T", [8, 128, C], dt.bfloat16, isOutput=False)
    b1c_d = nc.declare_dram_parameter("b1c", [128, 8], dt.float32, isOutput=False)
    crows_d = None
    if use_crows or use_b2row:
        crows_d = nc.declare_dram_parameter("crows", [1, 6 * C], dt.bfloat16, isOutput=False)
    out_d = nc.declare_dram_parameter("out", [n_rows, t_len, C], dt.float32, isOutput=True)

    with tile.TileContext(nc) as tc, ExitStack() as ctx:
        singles = ctx.enter_context(tc.tile_pool(name="singles", bufs=1))
        hb_pool = ctx.enter_context(tc.tile_pool(name="hb", bufs=1))
        xin = ctx.enter_context(tc.tile_pool(name="xin", bufs=12))
        x1p = ctx.enter_context(tc.tile_pool(name="x1p", bufs=16))
        hnorm = ctx.enter_context(tc.tile_pool(name="hnorm", bufs=8))
        stats = ctx.enter_context(tc.tile_pool(name="stats", bufs=16))
        h2b = ctx.enter_context(tc.tile_pool(name="h2b", bufs=2))
        ffb = ctx.enter_context(tc.tile_pool(name="ffb", bufs=2))
        outp = ctx.enter_context(tc.tile_pool(name="outp", bufs=8))
        tp_ps = ctx.enter_context(tc.tile_pool(name="tp_ps", bufs=2, space="PSUM"))
        acc_ps = ctx.enter_context(tc.tile_pool(name="acc_ps", bufs=3, space="PSUM"))
        up_ps = ctx.enter_context(tc.tile_pool(name="up_ps", bufs=3, space="PSUM"))

        # ---- constants / weights in SBUF ----
        ident = singles.tile([128, 128], dt.bfloat16)
        make_identity(nc, ident)
        eps_t = singles.tile([128, 1], dt.float32)
        nc.vector.memset(eps_t, EPS)
        waT = []
        for c in range(2):
            w = singles.tile([128, C], dt.bfloat16, tag=f"waT{c}")
            nc.gpsimd.dma_start(out=w, in_=waT_d[c])
            waT.append(w)
        wsT = []
        for s in range(4):
            row = []
            for c in range(2):
                w = singles.tile([128, C], dt.bfloat16, tag=f"wsT{s}{c}")
                nc.gpsimd.dma_start(out=w, in_=wsT_d[s, c])
                row.append(w)
            wsT.append(row)
        w1dr = singles.tile([128, 2, 4 * C], dt.float8e4, tag="w1dr")
        nc.gpsimd.dma_start(out=w1dr, in_=w1dr_d[:, :, :])
        w2T = []
        for fc in range(8):
            w = singles.tile([128, C], dt.bfloat16, tag=f"w2T{fc}")
            nc.gpsimd.dma_start(out=w, in_=w2T_d[fc])
            w2T.append(w)
        b1c = singles.tile([128, 8], dt.float32)
        nc.gpsimd.dma_start(out=b1c, in_=b1c_d[:, :])
        crows = None
        ones_row = None
        if crows_d is not None:
            crows = singles.tile([1, 6 * C], dt.bfloat16)
            nc.gpsimd.dma_start(out=crows, in_=crows_d[:, :])
            ones_row = singles.tile([1, PAD + t_len], dt.bfloat16)
            nc.vector.memset(ones_row, 1.0)
            nc.vector.memset(ones_row[:, 0:PAD], 0.0)

        def ln_stats(src_tile, mvW, k):
            st = stats.tile([128, 6], dt.float32, tag="st", name="st")
            nc.vector.bn_stats(st, src_tile)
            nc.vector.bn_aggr(mvW[:, 2 * k:2 * k + 2], st)

        def ln_batch_rsqrt(mvW):
            sdW = stats.tile([128, 4], dt.float32, tag="sd", name="sd")
            var_view = bass.AP(tensor=mvW.tensor, offset=mvW.offset + 1,
                               ap=[mvW.ap[0], [2, 4]])
            nc.scalar.activation(sdW, var_view, AF.Sqrt, bias=eps_t, scale=1.0)
            rsW = stats.tile([128, 4], dt.float32, tag="rs", name="rs")
            nc.vector.reciprocal(rsW, sdW)
            return rsW

        def ln_apply(src_tile, mvW, rsW, k, dst0, dst1, dst_dt):
            hn = hnorm.tile([128, C], dt.bfloat16, tag="hn", name="hn")
            nc.vector.tensor_scalar(
                out=hn, in0=src_tile, scalar1=mvW[:, 2 * k:2 * k + 1],
                scalar2=rsW[:, k:k + 1], op0=OP.subtract, op1=OP.mult,
            )
            for c, dst in enumerate((dst0, dst1)):
                pt = tp_ps.tile([128, 128], dt.bfloat16, tag="tp", name="tp")
                nc.tensor.transpose(pt, hn[:, 128 * c:128 * (c + 1)], ident)
                if c == 0:
                    nc.vector.tensor_copy(out=dst, in_=pt)
                else:
                    nc.scalar.copy(out=dst, in_=pt)

        state = {}

        def window_phase_a(slot, hB, r, w):
            t0w = w * WIN
            x_tiles, x1_tiles = [], []
            h2 = h2b.tile([128, 2, WIN], dt.float8e4, tag=f"h2dr{slot}",
                          name=f"h2dr{slot}")
            # LN1: batched stats then normalize+transpose into hB
            mv1 = stats.tile([128, 8], dt.float32, tag="mv1", name="mv1")
            for k in range(4):
                t0 = t0w + k * SUB
                xt = xin.tile([128, C], dt.float32, tag="x", name="x")
                nc.sync.dma_start(out=xt, in_=x_d[r, t0:t0 + SUB, :])
                x_tiles.append(xt)
                ln_stats(xt, mv1, k)
            rs1 = ln_batch_rsqrt(mv1)
            for k in range(4):
                col = PAD + t0w + k * SUB
                ln_apply(x_tiles[k], mv1, rs1, k,
                         hB[0][:, col:col + SUB], hB[1][:, col:col + SUB], None)
            # attention
            mv2 = stats.tile([128, 8], dt.float32, tag="mv2", name="mv2")
            for k in range(4):
                col = PAD + t0w + k * SUB
                ps = acc_ps.tile([128, C], dt.float32, tag="acc", name="sa")
                nc.tensor.matmul(ps, hB[0][:, col:col + SUB], waT[0],
                                 start=True, stop=False)
                nc.tensor.matmul(ps, hB[1][:, col:col + SUB], waT[1],
                                 start=False, stop=False)
                for s in range(1, 5):
                    for c in range(2):
                        last = (s == 4 and c == 1 and not use_crows)
                        nc.tensor.matmul(
                            ps, hB[c][:, col - s:col - s + SUB], wsT[s - 1][c],
                            start=False, stop=last,
                        )
                if use_crows:
                    nc.tensor.matmul(ps, ones_row[:, col:col + SUB], crows[:, 0:C],
                                     start=False, stop=False)
                    for s in range(1, 5):
                        nc.tensor.matmul(
                            ps, ones_row[:, col - s:col - s + SUB],
                            crows[:, s * C:(s + 1) * C], start=False, stop=(s == 4),
                        )
                x1t = x1p.tile([128, C], dt.float32, tag="x1", name="x1")
                nc.vector.tensor_add(out=x1t, in0=x_tiles[k], in1=ps)
                x1_tiles.append(x1t)
                ln_stats(x1t, mv2, k)
            # LN2 -> h2 (fp8) halves
            rs2 = ln_batch_rsqrt(mv2)
            for k in range(4):
                ln_apply(x1_tiles[k], mv2, rs2, k,
                         h2[:, 0, k * SUB:(k + 1) * SUB],
                         h2[:, 1, k * SUB:(k + 1) * SUB], None)
            # FFN up in fp8 DoubleRow + relu (scale compensates UP_W_SCALE)
            fftiles = []
            for fc in range(8):
                pu = up_ps.tile([128, WIN], dt.float32, tag="up", name="up")
                nc.tensor.matmul(pu, w1dr[:, :, 128 * fc:128 * (fc + 1)], h2,
                                 start=True, stop=True,
                                 perf_mode=mybir.MatmulPerfMode.DoubleRow)
                fb = ffb.tile([128, WIN], dt.bfloat16, tag=f"ffb{slot}{fc}",
                              name=f"ffb{slot}{fc}")
                nc.scalar.activation(fb, pu, AF.Relu, bias=b1c[:, fc:fc + 1],
                                     scale=1.0 / UP_W_SCALE)
                fftiles.append(fb)
            state[slot] = (x1_tiles, fftiles)

        def window_phase_b(slot, r, w):
            t0w = w * WIN
            x1_tiles, fftiles = state[slot]
            for k in range(4):
                t0 = t0w + k * SUB
                pd = acc_ps.tile([128, C], dt.float32, tag="acc", name="dn")
                for fc in range(8):
                    last = (fc == 7 and not use_b2row)
                    nc.tensor.matmul(pd, fftiles[fc][:, k * SUB:(k + 1) * SUB],
                                     w2T[fc], start=(fc == 0), stop=last)
                if use_b2row:
                    nc.tensor.matmul(pd, ones_row[:, PAD + t0:PAD + t0 + SUB],
                                     crows[:, 5 * C:6 * C], start=False, stop=True)
                ot = outp.tile([128, C], dt.float32, tag="o", name="o")
                nc.vector.tensor_add(out=ot, in0=x1_tiles[k], in1=pd)
                nc.sync.dma_start(out=out_d[r, t0:t0 + SUB, :], in_=ot)

        # Interleave pairs of batch rows: while one row's LN chains run on
        # DVE/ACT, the other row's matmuls keep the PE dense (HAM warm).
        nslots = min(2, n_rows)
        for rp in range(0, n_rows, nslots):
            hBs = []
            for slot in range(nslots):
                hB = []
                for c in range(2):
                    t_ = hb_pool.tile([128, PAD + t_len], dt.bfloat16,
                                      tag=f"hb{slot}{c}", name=f"hb{slot}{c}")
                    nc.gpsimd.memset(t_[:, 0:PAD], 0.0)
                    hB.append(t_)
                hBs.append(hB)
            for w in range(nwin):
                for slot in range(nslots):
                    window_phase_a(slot, hBs[slot], rp + slot, w)
                for slot in range(nslots):
                    window_phase_b(slot, rp + slot, w)

    nc.compile()
    return nc


_CACHE = {}


def _get_nc(key):
    if key not in _CACHE:
        _CACHE[key] = _build(use_crows=key[0], use_b2row=key[1])
    return _CACHE[key]


def _run(inputs, trace_dir=None):
    from concourse.bass_utils import run_bass_kernel_spmd
    from concourse import bass2jax

    x = np.asarray(inputs["x"], dtype=np.float32)
    w, use_crows, use_b2row = _prep(inputs)
    nc = _get_nc((use_crows, use_b2row))

    in_maps = []
    for core in range(NCORES):
        m = dict(w)
        m["x"] = np.ascontiguousarray(x[core * BPC:(core + 1) * BPC])
        in_maps.append(m)

    if trace_dir is None:
        res = run_bass_kernel_spmd(nc, in_maps, list(range(NCORES)))
        results, exec_ns = res.results, None
    else:
        from antenv.axon_hooks import get_axon_ntff_profile_hook

        hook = get_axon_ntff_profile_hook()
        os.makedirs(trace_dir, exist_ok=True)
        with hook(trace_dir, [0]):
            results = bass2jax.run_bass_via_pjrt(nc, in_maps, n_cores=NCORES)
        exec_ns = None  # caller post-processes the NTFFs

    out = np.concatenate([np.asarray(results[i]["out"]) for i in range(NCORES)], axis=0)
    return out, exec_ns


def kernel(**inputs):
    out, _ = _run(inputs)
    return out


# revision 22
# speedup vs baseline: 1.3377x; 1.3377x over previous
"""Trainium2 Bass kernel for nn_Block_627065225827 (dense_transformer).

Self-contained: hardcodes shapes B=32, T=4096, C=256, H=8 and the
data-parallel-over-batch sharding (4 batch rows per core, 8 cores).

Math (see reference):
    h   = LN1(x) * g1 + b1ln
    id  = h @ w_id.T ;  inf = h @ w_inf.T            (per-head view [H, hs])
    inf = inf / (1+K);  shifted[t] = inf[t - s_h]    (zero for t < s_h)
    sa  = (K/(1+K) * id + shifted) @ w_proj.T + b_proj
    x1  = x + sa
    ff  = relu(LN2(x1)*g2+b2ln @ w1.T + b1) @ w2.T + b2
    out = x1 + ff

Host-side algebraic folding (exact):
    sa[t] = w_a @ xhat[t] + sum_s W_s @ xhat[t-s] + const_rows
      w_a = w_proj @ (diag(a_row) @ (w_id * g1))           a_h = K/(1+K)
      W_s = w_proj[:, cols_s] @ ((w_inf * g1) * binv)[cols_s, :]
    so the per-head temporal shift becomes a free-dim offset into the
    transposed activation buffer hB (channels on partitions, tokens on
    free dim), with 4 zero pad columns implementing the t<s mask.
"""

import os
from contextlib import ExitStack

import numpy as np
import ml_dtypes

B, T, C, H = 32, 4096, 256, 8
HS = C // H
NCORES = 8
BPC = B // NCORES  # batch rows per core
SHIFTS = [1, 2, 3, 4, 1, 2, 3, 4]
EPS = 1e-5
PAD = 4  # zero columns at the head of hB for the shift mask
WIN = 512  # tokens per window
SUB = 128  # tokens per subtile (partition dim)

_f64 = np.float64
_bf16 = ml_dtypes.bfloat16
_fp8 = ml_dtypes.float8_e4m3
UP_W_SCALE = 16.0
FP8_UP = False


def _prep(inputs):
    """Fold LN gains/biases + per-head scalars into the weights (host, numpy)."""
    g = {k: np.asarray(v, dtype=_f64) for k, v in inputs.items() if k != "x"}
    K = np.exp(g["khead"])  # [H]
    a_row = np.repeat(K / (1.0 + K), HS)  # [C] per id-output channel
    b_row = np.repeat(1.0 / (1.0 + K), HS)  # [C] per inf-output channel

    w_id_g = g["w_id"] * g["ln1_g"][None, :]
    w_inf_g = g["w_inf"] * g["ln1_g"][None, :]
    w_id_s = w_id_g * a_row[:, None]
    w_inf_s = w_inf_g * b_row[:, None]

    w_a = g["w_proj"] @ w_id_s  # [C, C]
    wsT = np.zeros((4, C, C), _f64)
    c_s = np.zeros((4, C), _f64)
    cid = w_id_g @ g["ln1_b"]
    cinf = w_inf_g @ g["ln1_b"]
    for s in range(1, 5):
        cols = np.concatenate(
            [np.arange(h * HS, (h + 1) * HS) for h in range(H) if SHIFTS[h] == s]
        )
        wsT[s - 1] = (g["w_proj"][:, cols] @ w_inf_s[cols, :]).T
        c_s[s - 1] = g["w_proj"][:, cols] @ (b_row * cinf)[cols]
    c_a = g["w_proj"] @ (a_row * cid) + g["b_proj"]  # [C]

    w1_g = g["w1"] * g["ln2_g"][None, :]
    b1_eff = g["w1"] @ g["ln2_b"] + g["b1"]  # [4C]
    w2T = g["w2"].T  # [4C, C]
    b2_eff = g["b2"]  # [C]

    # fp8 up-projection: scale weights x16 (keeps N(0,.02) entries out of the
    # subnormal range), compensated exactly by relu's input scale (1/16).
    w1dr = (w1_g.T * UP_W_SCALE).reshape(2, 128, 4 * C).transpose(1, 0, 2)
    out = {
        "w_aT": np.ascontiguousarray(w_a.T.reshape(2, 128, C)).astype(_bf16),
        "wsT": np.ascontiguousarray(wsT.reshape(4, 2, 128, C)).astype(_bf16),
        "w1dr": np.ascontiguousarray(w1dr).astype(_fp8 if FP8_UP else _bf16),
        "w2T": np.ascontiguousarray(w2T.reshape(8, 128, C)).astype(_bf16),
        "b1c": np.ascontiguousarray(b1_eff.reshape(8, 128).T).astype(np.float32),
    }
    crows_attn = np.stack([c_a, c_s[0], c_s[1], c_s[2], c_s[3]])  # [5, C]
    use_crows = bool(np.abs(crows_attn).max() > 0)
    use_b2row = bool(np.abs(b2_eff).max() > 0)
    if use_crows or use_b2row:
        out["crows"] = np.concatenate([crows_attn, b2_eff[None, :]]).reshape(1, 6 * C).astype(_bf16)
    return out, use_crows, use_b2row


def _build(n_rows=BPC, t_len=T, use_crows=False, use_b2row=False):
    """Build + compile the per-core Bass program. Returns the finalized nc."""
    import concourse.bacc as bacc
    import concourse.bass as bass
    import concourse.mybir as mybir
    import concourse.tile as tile
    from concourse.masks import make_identity

    dt = mybir.dt
    AF = mybir.ActivationFunctionType
    OP = mybir.AluOpType

    nwin = t_len // WIN
    nc = bacc.Bacc("TRN2", target_bir_lowering=False, debug=False, num_devices=NCORES)

    x_d = nc.declare_dram_parameter("x", [n_rows, t_len, C], dt.float32, isOutput=False)
    waT_d = nc.declare_dram_parameter("w_aT", [2, 128, C], dt.bfloat16, isOutput=False)
    wsT_d = nc.declare_dram_parameter("wsT", [4, 2, 128, C], dt.bfloat16, isOutput=False)
    up_dt = dt.float8e4 if FP8_UP else dt.bfloat16
    w1dr_d = nc.declare_dram_parameter("w1dr", [128, 2, 4 * C], up_dt, isOutput=False)
    w2T_d = nc.declare_dram_parameter("w2T", [8, 128, C], dt.bfloat16, isOutput=False)
    b1c_d = nc.declare_dram_parameter("b1c", [128, 8], dt.float32, isOutput=False)
    crows_d = None
    if use_crows or use_b2row:
        crows_d = nc.declare_dram_parameter("crows", [1, 6 * C], dt.bfloat16, isOutput=False)
    out_d = nc.declare_dram_parameter("out", [n_rows, t_len, C], dt.float32, isOutput=True)

    with tile.TileContext(nc) as tc, ExitStack() as ctx:
        singles = ctx.enter_context(tc.tile_pool(name="singles", bufs=1))
        hb_pool = ctx.enter_context(tc.tile_pool(name="hb", bufs=1))
        xin = ctx.enter_context(tc.tile_pool(name="xin", bufs=20))
        x1p = ctx.enter_context(tc.tile_pool(name="x1p", bufs=16))
        hnorm = ctx.enter_context(tc.tile_pool(name="hnorm", bufs=8))
        stats = ctx.enter_context(tc.tile_pool(name="stats", bufs=16))
        h2b = ctx.enter_context(tc.tile_pool(name="h2b", bufs=2))
        ffb = ctx.enter_context(tc.tile_pool(name="ffb", bufs=2))
        outp = ctx.enter_context(tc.tile_pool(name="outp", bufs=8))
        tp_ps = ctx.enter_context(tc.tile_pool(name="tp_ps", bufs=3, space="PSUM"))
        acc_ps = ctx.enter_context(tc.tile_pool(name="acc_ps", bufs=3, space="PSUM"))
        up_ps = ctx.enter_context(tc.tile_pool(name="up_ps", bufs=2, space="PSUM"))

        # ---- constants / weights in SBUF ----
        ident = singles.tile([128, 128], dt.bfloat16)
        make_identity(nc, ident)
        eps_t = singles.tile([128, 1], dt.float32)
        nc.vector.memset(eps_t, EPS)
        waT = []
        for c in range(2):
            w = singles.tile([128, C], dt.bfloat16, tag=f"waT{c}")
            nc.gpsimd.dma_start(out=w, in_=waT_d[c])
            waT.append(w)
        wsT = []
        for s in range(4):
            row = []
            for c in range(2):
                w = singles.tile([128, C], dt.bfloat16, tag=f"wsT{s}{c}")
                nc.gpsimd.dma_start(out=w, in_=wsT_d[s, c])
                row.append(w)
            wsT.append(row)
        w1dr = singles.tile([128, 2, 4 * C], up_dt, tag="w1dr")
        nc.gpsimd.dma_start(out=w1dr, in_=w1dr_d[:, :, :])
        w2T = []
        for fc in range(8):
            w = singles.tile([128, C], dt.bfloat16, tag=f"w2T{fc}")
            nc.gpsimd.dma_start(out=w, in_=w2T_d[fc])
            w2T.append(w)
        b1c = singles.tile([128, 8], dt.float32)
        nc.gpsimd.dma_start(out=b1c, in_=b1c_d[:, :])
        crows = None
        ones_row = None
        if crows_d is not None:
            crows = singles.tile([1, 6 * C], dt.bfloat16)
            nc.gpsimd.dma_start(out=crows, in_=crows_d[:, :])
            ones_row = singles.tile([1, PAD + t_len], dt.bfloat16)
            nc.vector.memset(ones_row, 1.0)
            nc.vector.memset(ones_row[:, 0:PAD], 0.0)

        def ln_stats(src_tile, mvW, k):
            st = stats.tile([128, 6], dt.float32, tag="st", name="st")
            nc.vector.bn_stats(st, src_tile)
            nc.vector.bn_aggr(mvW[:, 2 * k:2 * k + 2], st)

        def ln_batch_rsqrt(mvW):
            sdW = stats.tile([128, 4], dt.float32, tag="sd", name="sd")
            var_view = bass.AP(tensor=mvW.tensor, offset=mvW.offset + 1,
                               ap=[mvW.ap[0], [2, 4]])
            nc.scalar.activation(sdW, var_view, AF.Sqrt, bias=eps_t, scale=1.0)
            rsW = stats.tile([128, 4], dt.float32, tag="rs", name="rs")
            nc.vector.reciprocal(rsW, sdW)
            return rsW

        def ln_apply(src_tile, mvW, rsW, k, dst0, dst1, dst_dt):
            hn = hnorm.tile([128, C], dt.bfloat16, tag="hn", name="hn")
            nc.gpsimd.tensor_scalar(
                out=hn, in0=src_tile, scalar1=mvW[:, 2 * k:2 * k + 1],
                scalar2=rsW[:, k:k + 1], op0=OP.subtract, op1=OP.mult,
            )
            pt = tp_ps.tile([128, 256], dt.float32, tag="tp", name="tp")
            for c, dst in enumerate((dst0, dst1)):
                nc.tensor.matmul(pt[:, 128 * c:128 * (c + 1)],
                                 hn[:, 128 * c:128 * (c + 1)], ident,
                                 start=True, stop=True)
                if c == 0 and dst.dtype == dt.bfloat16:
                    nc.vector.tensor_copy(out=dst, in_=pt[:, 0:128])
                else:
                    nc.scalar.copy(out=dst, in_=pt[:, 128 * c:128 * (c + 1)])

        state = {}

        def ln1_phase(slot, hB, r, w):
            t0w = w * WIN
            x_tiles = []
            mv1 = stats.tile([128, 8], dt.float32, tag="mv1", name="mv1")
            for k in range(4):
                t0 = t0w + k * SUB
                xt = xin.tile([128, C], dt.float32, tag="x", name="x")
                nc.sync.dma_start(out=xt, in_=x_d[r, t0:t0 + SUB, :])
                x_tiles.append(xt)
                ln_stats(xt, mv1, k)
            rs1 = ln_batch_rsqrt(mv1)
            for k in range(4):
                col = PAD + t0w + k * SUB
                ln_apply(x_tiles[k], mv1, rs1, k,
                         hB[0][:, col:col + SUB], hB[1][:, col:col + SUB], None)
            state[("x", slot)] = x_tiles

        def attn_phase(slot, hB, r, w):
            t0w = w * WIN
            x_tiles = state[("x", slot)]
            x1_tiles = []
            mv2 = stats.tile([128, 8], dt.float32, tag="mv2", name="mv2")
            for k in range(4):
                col = PAD + t0w + k * SUB
                ps = acc_ps.tile([128, C], dt.float32, tag="acc", name="sa")
                nc.tensor.matmul(ps, hB[0][:, col:col + SUB], waT[0],
                                 start=True, stop=False)
                nc.tensor.matmul(ps, hB[1][:, col:col + SUB], waT[1],
                                 start=False, stop=False)
                for s in range(1, 5):
                    for c in range(2):
                        last = (s == 4 and c == 1 and not use_crows)
                        nc.tensor.matmul(
                            ps, hB[c][:, col - s:col - s + SUB], wsT[s - 1][c],
                            start=False, stop=last,
                        )
                if use_crows:
                    nc.tensor.matmul(ps, ones_row[:, col:col + SUB], crows[:, 0:C],
                                     start=False, stop=False)
                    for s in range(1, 5):
                        nc.tensor.matmul(
                            ps, ones_row[:, col - s:col - s + SUB],
                            crows[:, s * C:(s + 1) * C], start=False, stop=(s == 4),
                        )
                x1t = x1p.tile([128, C], dt.float32, tag="x1", name="x1")
                nc.vector.tensor_add(out=x1t, in0=x_tiles[k], in1=ps)
                x1_tiles.append(x1t)
                ln_stats(x1t, mv2, k)
            state[("x1", slot, w)] = x1_tiles
            state[("mv2", slot)] = mv2

        def dn_group(slot, r, w, k, fftiles, x1_tiles):
            t0 = w * WIN + k * SUB
            pd = acc_ps.tile([128, C], dt.float32, tag="acc", name="dn")
            for fc in range(8):
                last = (fc == 7 and not use_b2row)
                nc.tensor.matmul(pd, fftiles[fc][:, k * SUB:(k + 1) * SUB],
                                 w2T[fc], start=(fc == 0), stop=last)
            if use_b2row:
                nc.tensor.matmul(pd, ones_row[:, PAD + t0:PAD + t0 + SUB],
                                 crows[:, 5 * C:6 * C], start=False, stop=True)
            ot = outp.tile([128, C], dt.float32, tag="o", name="o")
            nc.vector.tensor_add(out=ot, in0=x1_tiles[k], in1=pd)
            nc.sync.dma_start(out=out_d[r, t0:t0 + SUB, :], in_=ot)

        def up_phase(slot, r, w, prev):
            """LN2-apply + FFN-up for w, with dn groups of window w-1 interleaved
            between the relu-gated up matmuls to keep the PE stream dense."""
            x1_tiles = state[("x1", slot, w)]
            mv2 = state[("mv2", slot)]
            rs2 = ln_batch_rsqrt(mv2)
            h2 = h2b.tile([128, 2, WIN], up_dt, tag=f"h2dr{slot}",
                          name=f"h2dr{slot}")
            for k in range(4):
                ln_apply(x1_tiles[k], mv2, rs2, k,
                         h2[:, 0, k * SUB:(k + 1) * SUB],
                         h2[:, 1, k * SUB:(k + 1) * SUB], None)
            fftiles = []
            pv = prev  # (r_prev, w_prev, fftiles_prev, x1_prev) or None
            for fc in range(8):
                pu = up_ps.tile([128, WIN], dt.float32, tag="up", name="up")
                if FP8_UP:
                    nc.tensor.matmul(pu, w1dr[:, :, 128 * fc:128 * (fc + 1)], h2,
                                     start=True, stop=True,
                                     perf_mode=mybir.MatmulPerfMode.DoubleRow)
                else:
                    nc.tensor.matmul(pu, w1dr[:, 0, 128 * fc:128 * (fc + 1)],
                                     h2[:, 0, :], start=True, stop=False)
                    nc.tensor.matmul(pu, w1dr[:, 1, 128 * fc:128 * (fc + 1)],
                                     h2[:, 1, :], start=False, stop=True)
                fb = ffb.tile([128, WIN], dt.bfloat16, tag=f"ffb{slot}{fc}",
                              name=f"ffb{slot}{fc}")
                nc.scalar.activation(fb, pu, AF.Relu, bias=b1c[:, fc:fc + 1],
                                     scale=1.0 / UP_W_SCALE)
                fftiles.append(fb)
                if pv is not None and fc >= 2 and fc <= 5:
                    dn_group(slot, pv[0], pv[1], fc - 2, pv[2], pv[3])
            state[("ff", slot)] = (r, w, fftiles, x1_tiles)

        def dn_phase(slot, r, w):
            t0w = w * WIN
            x1_tiles, fftiles = state[slot]
            for k in range(4):
                t0 = t0w + k * SUB
                pd = acc_ps.tile([128, C], dt.float32, tag="acc", name="dn")
                for fc in range(8):
                    last = (fc == 7 and not use_b2row)
                    nc.tensor.matmul(pd, fftiles[fc][:, k * SUB:(k + 1) * SUB],
                                     w2T[fc], start=(fc == 0), stop=last)
                if use_b2row:
                    nc.tensor.matmul(pd, ones_row[:, PAD + t0:PAD + t0 + SUB],
                                     crows[:, 5 * C:6 * C], start=False, stop=True)
                ot = outp.tile([128, C], dt.float32, tag="o", name="o")
                nc.vector.tensor_add(out=ot, in0=x1_tiles[k], in1=pd)
                nc.sync.dma_start(out=out_d[r, t0:t0 + SUB, :], in_=ot)

        # Two batch rows interleaved; LN1 software-pipelined one window ahead;
        # dn(w-1) interleaved into up(w) to hide the relu chain.
        nslots = min(2, n_rows)
        for rp in range(0, n_rows, nslots):
            hBs = []
            for slot in range(nslots):
                hB = []
                for c in range(2):
                    t_ = hb_pool.tile([128, PAD + t_len], dt.bfloat16,
                                      tag=f"hb{slot}{c}", name=f"hb{slot}{c}")
                    nc.gpsimd.memset(t_[:, 0:PAD], 0.0)
                    hB.append(t_)
                hBs.append(hB)
            for slot in range(nslots):
                state[("ff", slot)] = None
                ln1_phase(slot, hBs[slot], rp + slot, 0)
                state[("xs", slot)] = state[("x", slot)]
            for w in range(nwin):
                for slot in range(nslots):
                    state[("x", slot)] = state[("xs", slot)]
                    attn_phase(slot, hBs[slot], rp + slot, w)
                for slot in range(nslots):
                    up_phase(slot, rp + slot, w, state[("ff", slot)])
                for slot in range(nslots):
                    if w + 1 < nwin:
                        ln1_phase(slot, hBs[slot], rp + slot, w + 1)
                        state[("xs", slot)] = state[("x", slot)]
            for slot in range(nslots):
                r_, w_, ff_, x1_ = state[("ff", slot)]
                for k in range(4):
                    dn_group(slot, r_, w_, k, ff_, x1_)
                state[("ff", slot)] = None

    nc.compile()
    return nc


_CACHE = {}


def _get_nc(key):
    if key not in _CACHE:
        _CACHE[key] = _build(use_crows=key[0], use_b2row=key[1])
    return _CACHE[key]


def _run(inputs, trace_dir=None):
    from concourse.bass_utils import run_bass_kernel_spmd
    from concourse import bass2jax

    x = np.asarray(inputs["x"], dtype=np.float32)
    w, use_crows, use_b2row = _prep(inputs)
    nc = _get_nc((use_crows, use_b2row))

    in_maps = []
    for core in range(NCORES):
        m = dict(w)
        m["x"] = np.ascontiguousarray(x[core * BPC:(core + 1) * BPC])
        in_maps.append(m)

    if trace_dir is None:
        res = run_bass_kernel_spmd(nc, in_maps, list(range(NCORES)))
        results, exec_ns = res.results, None
    else:
        from antenv.axon_hooks import get_axon_ntff_profile_hook

        hook = get_axon_ntff_profile_hook()
        os.makedirs(trace_dir, exist_ok=True)
        with hook(trace_dir, [0]):
            results = bass2jax.run_bass_via_pjrt(nc, in_maps, n_cores=NCORES)
        exec_ns = None  # caller post-processes the NTFFs

    out = np.concatenate([np.asarray(results[i]["out"]) for i in range(NCORES)], axis=0)
    return out, exec_ns


def kernel(**inputs):
    out, _ = _run(inputs)
    return out
